# revision 17
# baseline (speedup 1.0000x reference)
"""CTRNN kernel for Trainium2 (Bass/Tile), data-parallel over batch on 8 cores.

Reference computation (see nn_CTRNN):
    x_proj = einsum("tbi,hi->tbh", input, W_in) + b_in + b_hh          # [T,B,H]
    h_{t+1} = (1-a)*h_t + a*relu(x_proj_t + h_t @ W_hh.T),  a = 0.2
    returns (stacked h_t  [T,B,H],  final h  [B,H])

Device strategy (per core, B_local = 16):
  - Everything lives in the "hT" layout: hidden dim on partitions
    ([128 partitions, KT=4 k-tiles, 16 batch]), so the per-step matmul
    out = W_hh @ h keeps a layout-stable form:
        lhsT = W_hh^T block [128k, 128j] (stationary), rhs = hT k-tile [128, 16]
        psum_j [128, 16] accumulates over the 4 k-tiles.
  - x_proj is computed on-device in 8 windows of 128 steps:
    one [128i x 512j] x [i, 2048 cols] GEMM per window (cols = 128 steps * 16
    batch), PSUM copied to SBUF with the (b_in+b_hh) bias fused on ScalarE.
  - The 128 recurrence steps of a window are fully unrolled inside a
    tc.For_i window loop (body emitted once -> small NEFF, fast compile).
  - MMs are emitted k-major (k outer, j inner) with interleaved PSUM
    accumulation groups so the first MMs of step t+1 depend only on the
    earliest-finished chains of step t.
  - Host does all transposes/reshapes (free): inputs are pre-transposed,
    output is delivered as [T, 4, 128, 16] per core and re-assembled.
"""

import numpy as np

T, B, I, H = 1024, 128, 128, 512
NCORES = 8
BL = B // NCORES          # 16  per-core batch
KT = H // 128             # 4   128-row tiles of the hidden dim
ALPHA = 0.2

_module_cache = {}


def _build_module(mm_dtype="float32", T_=T, n_windows=8):
    import concourse.mybir as mybir
    from concourse import bacc
    from concourse.bass import ds
    from concourse.tile import TileContext

    f32 = mybir.dt.float32
    D = getattr(mybir.dt, mm_dtype)
    AFT = mybir.ActivationFunctionType
    ALU = mybir.AluOpType

    WS = T_ // n_windows       # steps per window
    WC = WS * BL               # input / x_proj columns per window

    nc = bacc.Bacc("TRN2")

    inT = nc.dram_tensor("inT", [I, (n_windows + 1) * WC], f32, kind="ExternalInput")
    winT = nc.dram_tensor("winT", [I, H], f32, kind="ExternalInput")
    whhT = nc.dram_tensor("whhT", [128, KT, H], D, kind="ExternalInput")
    biasd = nc.dram_tensor("biasd", [128, KT], f32, kind="ExternalInput")
    h0 = nc.dram_tensor("h0", [128, KT, BL], f32, kind="ExternalInput")
    out = nc.dram_tensor("out", [T_, KT, 128, BL], f32, kind="ExternalOutput")

    with TileContext(nc) as tc:
        with (
            tc.tile_pool(name="big", bufs=1) as big,
            tc.tile_pool(name="work", bufs=4) as work,
            tc.tile_pool(name="ps", bufs=8, space="PSUM") as ps,
        ):
            inT_sb = big.tile([I, (n_windows + 1) * WC], f32, name="inT_sb")
            winT_sb = big.tile([I, H], f32, name="winT_sb")
            whhT_sb = big.tile([128, KT, H], D, name="whhT_sb")
            bias_sb = big.tile([128, KT], f32, name="bias_sb")
            xp_sb = big.tile([128, KT, WC], f32, name="xp_sb")
            hm = [
                [big.tile([128, BL], f32, name=f"hm{p}_{k}") for k in range(KT)]
                for p in range(2)
            ]
            if D != f32:
                hd = [
                    [big.tile([128, BL], D, name=f"hd{p}_{k}") for k in range(KT)]
                    for p in range(2)
                ]
            else:
                hd = hm

            nc.sync.dma_start(winT_sb[:], winT[:])
            nc.sync.dma_start(whhT_sb[:], whhT[:])
            nc.sync.dma_start(bias_sb[:], biasd[:])
            for k in range(KT):
                nc.sync.dma_start(hm[0][k][:], h0[:, k])
                if D != f32:
                    nc.scalar.copy(hd[0][k][:], hm[0][k][:])
            nc.sync.dma_start(inT_sb[:, 0:WC], inT[:, 0:WC])

            CH = min(512, WC)  # x_proj free-dim chunk
            with tc.For_i(0, n_windows, 1) as w:
                base = w * WC
                # ---- x_proj GEMM for this window (+ bias, on ScalarE) ----
                for c in range(WC // CH):
                    for j in range(KT):
                        px = ps.tile([128, CH], f32, tag="ps", name=f"px{c}_{j}")
                        nc.tensor.matmul(
                            px[:],
                            winT_sb[:, j * 128 : (j + 1) * 128],
                            inT_sb[:, ds(base + c * CH, CH)],
                            start=True,
                            stop=True,
                        )
                        nc.scalar.activation(
                            xp_sb[:, j, c * CH : (c + 1) * CH],
                            px[:],
                            AFT.Identity,
                            bias=bias_sb[:, j : j + 1],
                            scale=1.0,
                        )
                # prefetch next window's input columns (last iter reads pad)
                nc.sync.dma_start(
                    inT_sb[:, ds(base + WC, WC)], inT[:, ds(base + WC, WC)]
                )
                # ---- recurrence: WS steps, fully unrolled ----
                for s in range(WS):
                    p, q = s % 2, 1 - (s % 2)
                    pj = [
                        ps.tile([128, BL], f32, tag="ps", name=f"pr{s % 2}_{j}")
                        for j in range(KT)
                    ]
                    for k in range(KT):
                        for j in range(KT):
                            nc.tensor.matmul(
                                pj[j][:],
                                whhT_sb[:, k, j * 128 : (j + 1) * 128],
                                hd[p][k][:],
                                start=(k == 0),
                                stop=(k == KT - 1),
                                skip_group_check=True,
                            )
                    for j in range(KT):
                        nc.vector.tensor_add(
                            pj[j][:], pj[j][:], xp_sb[:, j, s * BL : (s + 1) * BL]
                        )
                        r = work.tile([128, BL], f32, tag="r", name=f"r{s % 2}_{j}")
                        nc.scalar.activation(r[:], pj[j][:], AFT.Relu, scale=ALPHA)
                        nc.vector.scalar_tensor_tensor(
                            hm[q][j][:],
                            hm[p][j][:],
                            1.0 - ALPHA,
                            r[:],
                            ALU.mult,
                            ALU.add,
                        )
                        if D != f32:
                            nc.scalar.copy(hd[q][j][:], hm[q][j][:])
                        nc.sync.dma_start(out[w * WS + s, j], hm[q][j][:])
    nc.compile()
    return nc


def _build_module_v2(mm_dtype="float16", T_=T, ws=32):
    """16-bit fast path (v3 schedule).

    - alpha folded into W_in, W_hh, bias on host -> PSUM accumulates
      alpha*pre directly.
    - x_proj + bias are computed straight into PSUM: one tile per window
      set, [128, KT, 512] = 4 banks (one bank per j, zero-region aligned);
      per-step MMs accumulate on top. 2 sets ping-pong = 8 banks.
    - Blend+relu fused into ONE DVE op on the critical path:
          hd_new = max(ps, 0) + u,   u = 0.8*hd  (computed right after the
      blend, off the critical path, on the same engine).
    - The other set's x_proj MMs are split into N=128 chunks and drained
      one per step as PE filler during the blend wait.
    """
    import concourse.mybir as mybir
    from concourse import bacc
    from concourse.bass import ds
    from concourse.tile import TileContext

    f32 = mybir.dt.float32
    DD = getattr(mybir.dt, mm_dtype)
    ALU = mybir.AluOpType

    WC = ws * BL                     # psum columns per window
    PWC = max(WC, 512)               # pad so each j owns full banks
    nw_total = T_ // ws
    WPB = 8 if nw_total >= 8 else 2      # windows per loop body
    n_outer = T_ // (WPB * ws)
    assert n_outer * WPB * ws == T_ and ws % 2 == 0

    nc = bacc.Bacc("TRN2")

    inT = nc.dram_tensor("inT", [I, T_ * BL + WC], DD, kind="ExternalInput")
    winT = nc.dram_tensor("winT", [I, H], DD, kind="ExternalInput")
    whhT = nc.dram_tensor("whhT", [128, KT, H], DD, kind="ExternalInput")
    biasdg = nc.dram_tensor("biasdg", [128, KT, 128], DD, kind="ExternalInput")
    onesd = nc.dram_tensor("onesd", [I, WC], DD, kind="ExternalInput")
    h0 = nc.dram_tensor("h0", [128, KT, BL], f32, kind="ExternalInput")
    XCH = min(128, WC)               # x_proj chunk free dim
    n_xch = WC // XCH
    OC = min(8, ws)                  # steps per output DMA chunk
    assert ws % OC == 0
    out = nc.dram_tensor(
        "out", [T_ // OC, 128, KT, OC, BL], DD, kind="ExternalOutput"
    )

    with TileContext(nc) as tc:
        with (
            tc.tile_pool(name="big", bufs=1) as big,
            tc.tile_pool(name="ps", bufs=1, space="PSUM") as ps,
        ):
            inT_sb = big.tile([I, T_ * BL + WC], DD, name="inT_sb")
            winT_sb = big.tile([I, H], DD, name="winT_sb")
            whhT_sb = big.tile([128, KT, H], DD, name="whhT_sb")
            diag_sb = big.tile([128, KT, 128], DD, name="diag_sb")
            ones_sb = big.tile([I, WC], DD, name="ones_sb")
            h0_sb = big.tile([128, KT, BL], f32, name="h0_sb")
            hd = [big.tile([128, KT, BL], DD, name=f"hd{p}") for p in range(2)]
            u = [big.tile([128, KT, BL], DD, name=f"u{p}") for p in range(2)]
            stage = [
                big.tile([128, KT, OC, BL], DD, name=f"stage{c}") for c in range(2)
            ]
            px = [
                ps.tile([128, KT, PWC], f32, name=f"px{w}", tag=f"px{w}")
                for w in range(2)
            ]

            nc.sync.dma_start(inT_sb[:], inT[:])
            nc.sync.dma_start(winT_sb[:], winT[:])
            nc.sync.dma_start(whhT_sb[:], whhT[:])
            nc.sync.dma_start(diag_sb[:], biasdg[:])
            nc.sync.dma_start(ones_sb[:], onesd[:])
            nc.sync.dma_start(h0_sb[:], h0[:])
            nc.scalar.copy(hd[0][:], h0_sb[:])
            nc.vector.tensor_scalar_mul(u[0][:], hd[0][:], 1.0 - ALPHA)

            def xproj_mms(wset, col0):
                """Thunks filling window set `wset` from input col col0.
                Per j (bank): diag-bias chunks then x_proj chunks, all N=XCH
                so each fits a blend-wait hole. Only the first diag chunk
                uses start=True (pending-zero covers the whole bank)."""
                mms = []
                for j in range(KT):
                    for c in range(n_xch):
                        oc = px[wset][:, j, c * XCH : (c + 1) * XCH]
                        mms.append(
                            lambda oc=oc, j=j, c=c: nc.tensor.matmul(
                                oc, diag_sb[:, j], ones_sb[:, 0:XCH],
                                start=(c == 0), stop=False,
                                skip_group_check=True,
                            )
                        )
                    for c in range(n_xch):
                        oc = px[wset][:, j, c * XCH : (c + 1) * XCH]
                        mms.append(
                            lambda oc=oc, j=j, c=c: nc.tensor.matmul(
                                oc, winT_sb[:, j * 128 : (j + 1) * 128],
                                inT_sb[:, ds(col0 + c * XCH, XCH)],
                                start=False, stop=False, skip_group_check=True,
                            )
                        )
                return mms

            # prologue: window 0 into set 0
            for mm in xproj_mms(0, 0):
                mm()

            import concourse.mybir as _mb

            with tc.For_i(
                0, n_outer, 1, hint_engines=(_mb.EngineType.PE,)
            ) as w:
                for win in range(WPB):
                    # while running set win%2 (window WPB*w+win), fill the
                    # other set with the next window (last fill reads pad).
                    pend = xproj_mms((win + 1) % 2, (w * WPB + win + 1) * WC)
                    for s in range(ws):
                        p, q = s % 2, 1 - s % 2
                        for k in range(KT):
                            for j in range(KT):
                                nc.tensor.matmul(
                                    px[win % 2][:, j, s * BL : (s + 1) * BL],
                                    whhT_sb[:, k, j * 128 : (j + 1) * 128],
                                    hd[p][:, k, :],
                                    start=False,
                                    stop=(k == KT - 1),
                                    skip_group_check=True,
                                )
                        n_drain = len(pend) if s == ws - 1 else min(1, len(pend))
                        for _ in range(n_drain):
                            pend.pop(0)()
                        # hd_new = max(ps, 0) + u, split in k-halves so the
                        # next step's k=0,1 MMs overlap the second half;
                        # u = 0.8*hd pre-scaled off the critical path.
                        for g in range(2):
                            nc.vector.scalar_tensor_tensor(
                                hd[q][:, 2 * g : 2 * g + 2, :],
                                px[win % 2][:, 2 * g : 2 * g + 2,
                                            s * BL : (s + 1) * BL],
                                0.0,
                                u[p][:, 2 * g : 2 * g + 2, :],
                                ALU.max,
                                ALU.add,
                            )
                        for g in range(2):
                            nc.vector.tensor_scalar_mul(
                                u[q][:, 2 * g : 2 * g + 2, :],
                                hd[q][:, 2 * g : 2 * g + 2, :],
                                1.0 - ALPHA,
                            )
                        cpar = (win * ws + s) // OC % 2
                        nc.scalar.copy(stage[cpar][:, :, s % OC, :], hd[q][:])
                        if s % OC == OC - 1:
                            nc.sync.dma_start(
                                out[(w * WPB + win) * (ws // OC) + s // OC],
                                stage[cpar][:],
                            )
    nc.compile()
    return nc


def _host_prep(input, W_in, b_in, W_hh, b_hh, hidden0, mm_dtype, T_, n_windows):
    """Build the per-core in_maps (all host-side transposes happen here)."""
    import ml_dtypes

    WS = T_ // n_windows
    WC = WS * BL
    np_D = np.float32 if mm_dtype == "float32" else ml_dtypes.bfloat16

    input = np.asarray(input, np.float32)
    W_in = np.asarray(W_in, np.float32)
    W_hh = np.asarray(W_hh, np.float32)
    b = np.asarray(b_in, np.float32) + np.asarray(b_hh, np.float32)
    hidden0 = np.asarray(hidden0, np.float32)

    winT = np.ascontiguousarray(W_in.T)                       # [I, H]
    whhT = np.ascontiguousarray(
        W_hh.T.reshape(KT, 128, H).transpose(1, 0, 2).astype(np_D)
    )                                                          # [128, KT, H]
    bias = np.ascontiguousarray(b.reshape(KT, 128).T)          # [128, KT]

    in_maps = []
    for c in range(NCORES):
        b0 = c * BL
        x = input[:T_, b0 : b0 + BL, :]                        # [T_, BL, I]
        xt = np.ascontiguousarray(x.transpose(2, 0, 1).reshape(I, T_ * BL))
        inT = np.zeros((I, (n_windows + 1) * WC), np.float32)
        inT[:, : T_ * BL] = xt
        h0c = np.ascontiguousarray(
            hidden0[b0 : b0 + BL].T.reshape(KT, 128, BL).transpose(1, 0, 2)
        )                                                      # [128, KT, BL]
        in_maps.append(
            {
                "inT": inT,
                "winT": winT,
                "whhT": whhT,
                "biasd": bias,
                "h0": h0c,
            }
        )
    return in_maps


def _host_prep_v2(input, W_in, b_in, W_hh, b_hh, hidden0, mm_dtype, T_, ws):
    import ml_dtypes

    WC = ws * BL
    np_D = np.float16 if mm_dtype == "float16" else ml_dtypes.bfloat16

    input = np.asarray(input, np.float32)
    W_in = np.asarray(W_in, np.float32) * ALPHA
    W_hh = np.asarray(W_hh, np.float32) * ALPHA
    b = (np.asarray(b_in, np.float32) + np.asarray(b_hh, np.float32)) * ALPHA
    hidden0 = np.asarray(hidden0, np.float32)

    winT = np.ascontiguousarray(W_in.T.astype(np_D))           # [I, H]
    whhT = np.ascontiguousarray(
        W_hh.T.reshape(KT, 128, H).transpose(1, 0, 2).astype(np_D)
    )                                                          # [128, KT, H]
    # diag(b j-slice) per j-tile: biasdg[p, j, m] = b[j*128+p] if m==p else 0
    biasdg = np.zeros((128, KT, 128), np.float32)
    bt = b.reshape(KT, 128)
    for j in range(KT):
        np.fill_diagonal(biasdg[:, j, :], bt[j])
    biasdg = biasdg.astype(np_D)
    ones = np.ones((I, WC), np_D)

    in_maps = []
    for c in range(NCORES):
        b0 = c * BL
        x = input[:T_, b0 : b0 + BL, :]                        # [T_, BL, I]
        xt = x.transpose(2, 0, 1).reshape(I, T_ * BL)
        inT = np.zeros((I, T_ * BL + WC), np_D)
        inT[:, : T_ * BL] = xt
        h0c = np.ascontiguousarray(
            hidden0[b0 : b0 + BL].T.reshape(KT, 128, BL).transpose(1, 0, 2)
        )                                                      # [128, KT, BL]
        in_maps.append(
            {
                "inT": inT,
                "winT": winT,
                "whhT": whhT,
                "biasdg": biasdg,
                "onesd": ones,
                "h0": h0c,
            }
        )
    return in_maps


def _unshard(results, T_):
    """v1 results: {'out': [T_, KT, 128, BL]} per core -> (output, hidden)."""
    outs = []
    for res in results:
        o = np.asarray(res["out"])                             # [T_, KT, 128, BL]
        outs.append(o.transpose(0, 3, 1, 2).reshape(T_, BL, H))
    output = np.concatenate(outs, axis=1)                      # [T_, B, H]
    return output, output[-1].copy()


def _unshard_v2(results, T_):
    """v2 results: {'out': [T_/OC, 128, KT, OC, BL]} -> (output, hidden) f32."""
    outs = []
    for res in results:
        o = np.asarray(res["out"]).astype(np.float32)   # [T/OC, 128, KT, OC, BL]
        outs.append(o.transpose(0, 3, 4, 2, 1).reshape(T_, BL, H))
    output = np.concatenate(outs, axis=1)               # [T_, B, H]
    return output, output[-1].copy()


def run(input, W_in, b_in, W_hh, b_hh, hidden0, mm_dtype="float16", T_=T,
        n_windows=8, ws=32, trace=False):
    from concourse.bass_utils import run_bass_kernel_spmd

    v2 = mm_dtype in ("float16", "bfloat16")
    key = (mm_dtype, T_, n_windows, ws)
    if key not in _module_cache:
        _module_cache[key] = (
            _build_module_v2(mm_dtype, T_, ws)
            if v2
            else _build_module(mm_dtype, T_, n_windows)
        )
    nc = _module_cache[key]

    if v2:
        in_maps = _host_prep_v2(
            input, W_in, b_in, W_hh, b_hh, hidden0, mm_dtype, T_, ws
        )
    else:
        in_maps = _host_prep(
            input, W_in, b_in, W_hh, b_hh, hidden0, mm_dtype, T_, n_windows
        )
    res = run_bass_kernel_spmd(
        nc, in_maps, core_ids=list(range(NCORES)), trace=trace
    )
    output, hidden = (_unshard_v2 if v2 else _unshard)(res.results, T_)
    return (output, hidden), res


def kernel(input, W_in, b_in, W_hh, b_hh, hidden0):
    (output, hidden), _ = run(input, W_in, b_in, W_hh, b_hh, hidden0)
    return output, hidden


# revision 18
# speedup vs baseline: 1.1836x; 1.1836x over previous
"""CTRNN kernel for Trainium2 (Bass/Tile), data-parallel over batch on 8 cores.

Reference computation (see nn_CTRNN):
    x_proj = einsum("tbi,hi->tbh", input, W_in) + b_in + b_hh          # [T,B,H]
    h_{t+1} = (1-a)*h_t + a*relu(x_proj_t + h_t @ W_hh.T),  a = 0.2
    returns (stacked h_t  [T,B,H],  final h  [B,H])

Device strategy (per core, B_local = 16):
  - Everything lives in the "hT" layout: hidden dim on partitions
    ([128 partitions, KT=4 k-tiles, 16 batch]), so the per-step matmul
    out = W_hh @ h keeps a layout-stable form:
        lhsT = W_hh^T block [128k, 128j] (stationary), rhs = hT k-tile [128, 16]
        psum_j [128, 16] accumulates over the 4 k-tiles.
  - x_proj is computed on-device in 8 windows of 128 steps:
    one [128i x 512j] x [i, 2048 cols] GEMM per window (cols = 128 steps * 16
    batch), PSUM copied to SBUF with the (b_in+b_hh) bias fused on ScalarE.
  - The 128 recurrence steps of a window are fully unrolled inside a
    tc.For_i window loop (body emitted once -> small NEFF, fast compile).
  - MMs are emitted k-major (k outer, j inner) with interleaved PSUM
    accumulation groups so the first MMs of step t+1 depend only on the
    earliest-finished chains of step t.
  - Host does all transposes/reshapes (free): inputs are pre-transposed,
    output is delivered as [T, 4, 128, 16] per core and re-assembled.
"""

import numpy as np

T, B, I, H = 1024, 128, 128, 512
NCORES = 8
BL = B // NCORES          # 16  per-core batch
KT = H // 128             # 4   128-row tiles of the hidden dim
ALPHA = 0.2

_module_cache = {}


def _build_module(mm_dtype="float32", T_=T, n_windows=8):
    import concourse.mybir as mybir
    from concourse import bacc
    from concourse.bass import ds
    from concourse.tile import TileContext

    f32 = mybir.dt.float32
    D = getattr(mybir.dt, mm_dtype)
    AFT = mybir.ActivationFunctionType
    ALU = mybir.AluOpType

    WS = T_ // n_windows       # steps per window
    WC = WS * BL               # input / x_proj columns per window

    nc = bacc.Bacc("TRN2")

    inT = nc.dram_tensor("inT", [I, (n_windows + 1) * WC], f32, kind="ExternalInput")
    winT = nc.dram_tensor("winT", [I, H], f32, kind="ExternalInput")
    whhT = nc.dram_tensor("whhT", [128, KT, H], D, kind="ExternalInput")
    biasd = nc.dram_tensor("biasd", [128, KT], f32, kind="ExternalInput")
    h0 = nc.dram_tensor("h0", [128, KT, BL], f32, kind="ExternalInput")
    out = nc.dram_tensor("out", [T_, KT, 128, BL], f32, kind="ExternalOutput")

    with TileContext(nc) as tc:
        with (
            tc.tile_pool(name="big", bufs=1) as big,
            tc.tile_pool(name="work", bufs=4) as work,
            tc.tile_pool(name="ps", bufs=8, space="PSUM") as ps,
        ):
            inT_sb = big.tile([I, (n_windows + 1) * WC], f32, name="inT_sb")
            winT_sb = big.tile([I, H], f32, name="winT_sb")
            whhT_sb = big.tile([128, KT, H], D, name="whhT_sb")
            bias_sb = big.tile([128, KT], f32, name="bias_sb")
            xp_sb = big.tile([128, KT, WC], f32, name="xp_sb")
            hm = [
                [big.tile([128, BL], f32, name=f"hm{p}_{k}") for k in range(KT)]
                for p in range(2)
            ]
            if D != f32:
                hd = [
                    [big.tile([128, BL], D, name=f"hd{p}_{k}") for k in range(KT)]
                    for p in range(2)
                ]
            else:
                hd = hm

            nc.sync.dma_start(winT_sb[:], winT[:])
            nc.sync.dma_start(whhT_sb[:], whhT[:])
            nc.sync.dma_start(bias_sb[:], biasd[:])
            for k in range(KT):
                nc.sync.dma_start(hm[0][k][:], h0[:, k])
                if D != f32:
                    nc.scalar.copy(hd[0][k][:], hm[0][k][:])
            nc.sync.dma_start(inT_sb[:, 0:WC], inT[:, 0:WC])

            CH = min(512, WC)  # x_proj free-dim chunk
            with tc.For_i(0, n_windows, 1) as w:
                base = w * WC
                # ---- x_proj GEMM for this window (+ bias, on ScalarE) ----
                for c in range(WC // CH):
                    for j in range(KT):
                        px = ps.tile([128, CH], f32, tag="ps", name=f"px{c}_{j}")
                        nc.tensor.matmul(
                            px[:],
                            winT_sb[:, j * 128 : (j + 1) * 128],
                            inT_sb[:, ds(base + c * CH, CH)],
                            start=True,
                            stop=True,
                        )
                        nc.scalar.activation(
                            xp_sb[:, j, c * CH : (c + 1) * CH],
                            px[:],
                            AFT.Identity,
                            bias=bias_sb[:, j : j + 1],
                            scale=1.0,
                        )
                # prefetch next window's input columns (last iter reads pad)
                nc.sync.dma_start(
                    inT_sb[:, ds(base + WC, WC)], inT[:, ds(base + WC, WC)]
                )
                # ---- recurrence: WS steps, fully unrolled ----
                for s in range(WS):
                    p, q = s % 2, 1 - (s % 2)
                    pj = [
                        ps.tile([128, BL], f32, tag="ps", name=f"pr{s % 2}_{j}")
                        for j in range(KT)
                    ]
                    for k in range(KT):
                        for j in range(KT):
                            nc.tensor.matmul(
                                pj[j][:],
                                whhT_sb[:, k, j * 128 : (j + 1) * 128],
                                hd[p][k][:],
                                start=(k == 0),
                                stop=(k == KT - 1),
                                skip_group_check=True,
                            )
                    for j in range(KT):
                        nc.vector.tensor_add(
                            pj[j][:], pj[j][:], xp_sb[:, j, s * BL : (s + 1) * BL]
                        )
                        r = work.tile([128, BL], f32, tag="r", name=f"r{s % 2}_{j}")
                        nc.scalar.activation(r[:], pj[j][:], AFT.Relu, scale=ALPHA)
                        nc.vector.scalar_tensor_tensor(
                            hm[q][j][:],
                            hm[p][j][:],
                            1.0 - ALPHA,
                            r[:],
                            ALU.mult,
                            ALU.add,
                        )
                        if D != f32:
                            nc.scalar.copy(hd[q][j][:], hm[q][j][:])
                        nc.sync.dma_start(out[w * WS + s, j], hm[q][j][:])
    nc.compile()
    return nc


def _build_module_v2(mm_dtype="float16", T_=T, ws=32):
    """16-bit fast path (v3 schedule).

    - alpha folded into W_in, W_hh, bias on host -> PSUM accumulates
      alpha*pre directly.
    - x_proj + bias are computed straight into PSUM: one tile per window
      set, [128, KT, 512] = 4 banks (one bank per j, zero-region aligned);
      per-step MMs accumulate on top. 2 sets ping-pong = 8 banks.
    - Blend+relu fused into ONE DVE op on the critical path:
          hd_new = max(ps, 0) + u,   u = 0.8*hd  (computed right after the
      blend, off the critical path, on the same engine).
    - The other set's x_proj MMs are split into N=128 chunks and drained
      one per step as PE filler during the blend wait.
    """
    import concourse.mybir as mybir
    from concourse import bacc
    from concourse.bass import ds
    from concourse.tile import TileContext

    f32 = mybir.dt.float32
    DD = getattr(mybir.dt, mm_dtype)
    ALU = mybir.AluOpType

    WC = ws * BL                     # psum columns per window
    PWC = max(WC, 512)               # pad so each j owns full banks
    nw_total = T_ // ws
    WPB = 8 if nw_total >= 8 else 2      # windows per loop body
    n_outer = T_ // (WPB * ws)
    assert n_outer * WPB * ws == T_ and ws % 2 == 0

    nc = bacc.Bacc("TRN2")

    inT = nc.dram_tensor("inT", [I, T_ * BL + WC], DD, kind="ExternalInput")
    winT = nc.dram_tensor("winT", [I, H], DD, kind="ExternalInput")
    whhT = nc.dram_tensor("whhT", [128, KT, H], DD, kind="ExternalInput")
    biasdg = nc.dram_tensor("biasdg", [128, KT, 128], DD, kind="ExternalInput")
    onesd = nc.dram_tensor("onesd", [I, WC], DD, kind="ExternalInput")
    h0 = nc.dram_tensor("h0", [128, KT, BL], f32, kind="ExternalInput")
    XCH = min(128, WC)               # x_proj chunk free dim
    n_xch = WC // XCH
    OC = min(8, ws)                  # steps per output DMA chunk
    assert ws % OC == 0
    out = nc.dram_tensor(
        "out", [T_ // OC, 128, KT, OC, BL], DD, kind="ExternalOutput"
    )

    with TileContext(nc) as tc:
        with (
            tc.tile_pool(name="big", bufs=1) as big,
            tc.tile_pool(name="ps", bufs=1, space="PSUM") as ps,
        ):
            inT_sb = big.tile([I, T_ * BL + WC], DD, name="inT_sb")
            winT_sb = big.tile([I, H], DD, name="winT_sb")
            whhT_sb = big.tile([128, KT, H], DD, name="whhT_sb")
            diag_sb = big.tile([128, KT, 128], DD, name="diag_sb")
            ones_sb = big.tile([I, WC], DD, name="ones_sb")
            h0_sb = big.tile([128, KT, BL], f32, name="h0_sb")
            hd = [big.tile([128, KT, BL], DD, name=f"hd{p}") for p in range(2)]
            u = [big.tile([128, KT, BL], DD, name=f"u{p}") for p in range(2)]
            stage = [
                big.tile([128, KT, OC, BL], DD, name=f"stage{c}") for c in range(2)
            ]
            px = [
                ps.tile([128, KT, PWC], f32, name=f"px{w}", tag=f"px{w}")
                for w in range(2)
            ]

            nc.sync.dma_start(inT_sb[:], inT[:])
            nc.sync.dma_start(winT_sb[:], winT[:])
            nc.sync.dma_start(whhT_sb[:], whhT[:])
            nc.sync.dma_start(diag_sb[:], biasdg[:])
            nc.sync.dma_start(ones_sb[:], onesd[:])
            nc.sync.dma_start(h0_sb[:], h0[:])
            nc.scalar.copy(hd[0][:], h0_sb[:])
            nc.vector.tensor_scalar_mul(u[0][:], hd[0][:], 1.0 - ALPHA)

            def xproj_mms(wset, col0):
                """Thunks filling window set `wset` from input col col0.
                Per j (bank): diag-bias chunks then x_proj chunks, all N=XCH
                so each fits a blend-wait hole. Only the first diag chunk
                uses start=True (pending-zero covers the whole bank)."""
                mms = []
                for j in range(KT):
                    for c in range(n_xch):
                        oc = px[wset][:, j, c * XCH : (c + 1) * XCH]
                        mms.append(
                            lambda oc=oc, j=j, c=c: nc.tensor.matmul(
                                oc, diag_sb[:, j], ones_sb[:, 0:XCH],
                                start=(c == 0), stop=False,
                                skip_group_check=True,
                            )
                        )
                    for c in range(n_xch):
                        oc = px[wset][:, j, c * XCH : (c + 1) * XCH]
                        mms.append(
                            lambda oc=oc, j=j, c=c: nc.tensor.matmul(
                                oc, winT_sb[:, j * 128 : (j + 1) * 128],
                                inT_sb[:, ds(col0 + c * XCH, XCH)],
                                start=False, stop=False, skip_group_check=True,
                            )
                        )
                return mms

            # prologue: window 0 into set 0
            for mm in xproj_mms(0, 0):
                mm()

            for w in range(n_outer):
                for win in range(WPB):
                    # while running set win%2 (window WPB*w+win), fill the
                    # other set with the next window (last fill reads pad).
                    pend = xproj_mms((win + 1) % 2, (w * WPB + win + 1) * WC)
                    for s in range(ws):
                        p, q = s % 2, 1 - s % 2
                        for k in range(KT):
                            for j in range(KT):
                                nc.tensor.matmul(
                                    px[win % 2][:, j, s * BL : (s + 1) * BL],
                                    whhT_sb[:, k, j * 128 : (j + 1) * 128],
                                    hd[p][:, k, :],
                                    start=False,
                                    stop=(k == KT - 1),
                                    skip_group_check=True,
                                )
                        n_drain = len(pend) if s == ws - 1 else min(1, len(pend))
                        for _ in range(n_drain):
                            pend.pop(0)()
                        # hd_new = max(ps, 0) + u (one DVE op on the
                        # critical path); u = 0.8*hd pre-scaled off it.
                        nc.vector.scalar_tensor_tensor(
                            hd[q][:],
                            px[win % 2][:, :, s * BL : (s + 1) * BL],
                            0.0,
                            u[p][:],
                            ALU.max,
                            ALU.add,
                        )
                        nc.vector.tensor_scalar_mul(
                            u[q][:], hd[q][:], 1.0 - ALPHA
                        )
                        cpar = (win * ws + s) // OC % 2
                        nc.scalar.copy(stage[cpar][:, :, s % OC, :], hd[q][:])
                        if s % OC == OC - 1:
                            nc.sync.dma_start(
                                out[(w * WPB + win) * (ws // OC) + s // OC],
                                stage[cpar][:],
                            )
    nc.compile()
    return nc


def _host_prep(input, W_in, b_in, W_hh, b_hh, hidden0, mm_dtype, T_, n_windows):
    """Build the per-core in_maps (all host-side transposes happen here)."""
    import ml_dtypes

    WS = T_ // n_windows
    WC = WS * BL
    np_D = np.float32 if mm_dtype == "float32" else ml_dtypes.bfloat16

    input = np.asarray(input, np.float32)
    W_in = np.asarray(W_in, np.float32)
    W_hh = np.asarray(W_hh, np.float32)
    b = np.asarray(b_in, np.float32) + np.asarray(b_hh, np.float32)
    hidden0 = np.asarray(hidden0, np.float32)

    winT = np.ascontiguousarray(W_in.T)                       # [I, H]
    whhT = np.ascontiguousarray(
        W_hh.T.reshape(KT, 128, H).transpose(1, 0, 2).astype(np_D)
    )                                                          # [128, KT, H]
    bias = np.ascontiguousarray(b.reshape(KT, 128).T)          # [128, KT]

    in_maps = []
    for c in range(NCORES):
        b0 = c * BL
        x = input[:T_, b0 : b0 + BL, :]                        # [T_, BL, I]
        xt = np.ascontiguousarray(x.transpose(2, 0, 1).reshape(I, T_ * BL))
        inT = np.zeros((I, (n_windows + 1) * WC), np.float32)
        inT[:, : T_ * BL] = xt
        h0c = np.ascontiguousarray(
            hidden0[b0 : b0 + BL].T.reshape(KT, 128, BL).transpose(1, 0, 2)
        )                                                      # [128, KT, BL]
        in_maps.append(
            {
                "inT": inT,
                "winT": winT,
                "whhT": whhT,
                "biasd": bias,
                "h0": h0c,
            }
        )
    return in_maps


def _host_prep_v2(input, W_in, b_in, W_hh, b_hh, hidden0, mm_dtype, T_, ws):
    import ml_dtypes

    WC = ws * BL
    np_D = np.float16 if mm_dtype == "float16" else ml_dtypes.bfloat16

    input = np.asarray(input, np.float32)
    W_in = np.asarray(W_in, np.float32) * ALPHA
    W_hh = np.asarray(W_hh, np.float32) * ALPHA
    b = (np.asarray(b_in, np.float32) + np.asarray(b_hh, np.float32)) * ALPHA
    hidden0 = np.asarray(hidden0, np.float32)

    winT = np.ascontiguousarray(W_in.T.astype(np_D))           # [I, H]
    whhT = np.ascontiguousarray(
        W_hh.T.reshape(KT, 128, H).transpose(1, 0, 2).astype(np_D)
    )                                                          # [128, KT, H]
    # diag(b j-slice) per j-tile: biasdg[p, j, m] = b[j*128+p] if m==p else 0
    biasdg = np.zeros((128, KT, 128), np.float32)
    bt = b.reshape(KT, 128)
    for j in range(KT):
        np.fill_diagonal(biasdg[:, j, :], bt[j])
    biasdg = biasdg.astype(np_D)
    ones = np.ones((I, WC), np_D)

    in_maps = []
    for c in range(NCORES):
        b0 = c * BL
        x = input[:T_, b0 : b0 + BL, :]                        # [T_, BL, I]
        xt = x.transpose(2, 0, 1).reshape(I, T_ * BL)
        inT = np.zeros((I, T_ * BL + WC), np_D)
        inT[:, : T_ * BL] = xt
        h0c = np.ascontiguousarray(
            hidden0[b0 : b0 + BL].T.reshape(KT, 128, BL).transpose(1, 0, 2)
        )                                                      # [128, KT, BL]
        in_maps.append(
            {
                "inT": inT,
                "winT": winT,
                "whhT": whhT,
                "biasdg": biasdg,
                "onesd": ones,
                "h0": h0c,
            }
        )
    return in_maps


def _unshard(results, T_):
    """v1 results: {'out': [T_, KT, 128, BL]} per core -> (output, hidden)."""
    outs = []
    for res in results:
        o = np.asarray(res["out"])                             # [T_, KT, 128, BL]
        outs.append(o.transpose(0, 3, 1, 2).reshape(T_, BL, H))
    output = np.concatenate(outs, axis=1)                      # [T_, B, H]
    return output, output[-1].copy()


def _unshard_v2(results, T_):
    """v2 results: {'out': [T_/OC, 128, KT, OC, BL]} -> (output, hidden) f32."""
    outs = []
    for res in results:
        o = np.asarray(res["out"]).astype(np.float32)   # [T/OC, 128, KT, OC, BL]
        outs.append(o.transpose(0, 3, 4, 2, 1).reshape(T_, BL, H))
    output = np.concatenate(outs, axis=1)               # [T_, B, H]
    return output, output[-1].copy()


def run(input, W_in, b_in, W_hh, b_hh, hidden0, mm_dtype="float16", T_=T,
        n_windows=8, ws=32, trace=False):
    from concourse.bass_utils import run_bass_kernel_spmd

    v2 = mm_dtype in ("float16", "bfloat16")
    key = (mm_dtype, T_, n_windows, ws)
    if key not in _module_cache:
        _module_cache[key] = (
            _build_module_v2(mm_dtype, T_, ws)
            if v2
            else _build_module(mm_dtype, T_, n_windows)
        )
    nc = _module_cache[key]

    if v2:
        in_maps = _host_prep_v2(
            input, W_in, b_in, W_hh, b_hh, hidden0, mm_dtype, T_, ws
        )
    else:
        in_maps = _host_prep(
            input, W_in, b_in, W_hh, b_hh, hidden0, mm_dtype, T_, n_windows
        )
    res = run_bass_kernel_spmd(
        nc, in_maps, core_ids=list(range(NCORES)), trace=trace
    )
    output, hidden = (_unshard_v2 if v2 else _unshard)(res.results, T_)
    return (output, hidden), res


def kernel(input, W_in, b_in, W_hh, b_hh, hidden0):
    (output, hidden), _ = run(input, W_in, b_in, W_hh, b_hh, hidden0)
    return output, hidden
